# revision 3
# baseline (speedup 1.0000x reference)
"""Karras optimal denoiser (kNN softmax over train set) on 8 trn2 cores, v2.

out[b] = sum_n softmax_n(-0.5*||x_b - y_n||^2 / sigma_b^2) * y_n

Single-pass flash-style kernel per core over its N-shard:
  U[n,b]  = y_n . x_b - 0.5*||y_n||^2      (fp32 GEMM, transposed orient,
                                            full 128-wide PE via aug row)
  online softmax over supers of 4 n-chunks (max/rescale merge)
  acc     = W^T-weighted sum of Y          (bf16 GEMM, [64,512] psum tiles)

Y ships once per layout: yt fp32 tiled [128p][49kk][25k][128j] for phase 1,
yb bf16 row-permuted [128p][49kk][3072d] for phase 3. All DMAs are 128
contiguous per-partition runs (no descriptor storms).
Host merges per-core (m, s, acc) with a logsumexp merge.
"""

import math
import sys

sys.path.insert(0, "/opt/trn_rl_repo")

import numpy as np

B = 64
CC, HH, WW = 3, 32, 32
D = CC * HH * WW          # 3072
N = 50000
NCORES = 8

NSH = N // NCORES         # 6250 rows per core
NP = 6272                 # padded shard: 49*128
NK = NP // 128            # 49 n-chunks
DP = 3200                 # 3072 + aug row + pad = 25*128
KCH = DP // 128           # 25 k-chunks
DG = D // 512             # 6 psum column groups
SUP = 4                   # chunks per super (flash block)
Y2_PAD = 1.0e8            # pad rows get huge ||y||^2 -> weight 0

_PROGRAM_CACHE: dict = {}


def _supers():
    out = []
    c0 = 0
    while c0 < NK:
        out.append((c0, min(SUP, NK - c0)))
        c0 += SUP
    return out


def _build_program():
    import concourse.bacc as bacc
    import concourse.mybir as mybir
    import concourse.tile as tile
    from concourse.bass import ts, ds

    f32 = mybir.dt.float32
    bf16 = mybir.dt.bfloat16
    Exp = mybir.ActivationFunctionType.Exp
    AXY = mybir.AxisListType.XY
    mx = mybir.AluOpType.max
    mult = mybir.AluOpType.mult
    add = mybir.AluOpType.add
    sub = mybir.AluOpType.subtract

    nc = bacc.Bacc()
    xt_d = nc.declare_dram_parameter("xt", [128, KCH * B], f32, isOutput=False)
    iv_d = nc.declare_dram_parameter("iv", [B], f32, isOutput=False)
    id_d = nc.declare_dram_parameter("ident", [128, 128], f32, isOutput=False)
    yt_d = nc.declare_dram_parameter(
        "yt", [128, NK * KCH * 128], f32, isOutput=False
    )
    yb_d = nc.declare_dram_parameter("yb", [128, NK * D], bf16, isOutput=False)
    out_d = nc.declare_dram_parameter("out", [B, D + 2], f32, isOutput=True)

    xt_v = xt_d.rearrange("p (k b) -> p k b", k=KCH)          # [128,25,64]
    yt_v = yt_d.rearrange("p (kk k j) -> p kk k j", kk=NK, k=KCH)
    yb_v = yb_d.rearrange("p (kk d) -> p kk d", kk=NK)

    supers = _supers()
    n_sup = len(supers)

    with tile.TileContext(nc) as tc:
        with (
            tc.tile_pool(name="const", bufs=1) as constp,
            tc.tile_pool(name="yt", bufs=2) as ytp,
            tc.tile_pool(name="yb", bufs=2) as ybp,
            tc.tile_pool(name="lw", bufs=2) as lwp,
            tc.tile_pool(name="wt", bufs=3) as wtp,
            tc.tile_pool(name="st", bufs=1) as stp,
            tc.tile_pool(name="accps", bufs=1, space="PSUM") as accpsp,
            tc.tile_pool(name="tp", bufs=2, space="PSUM") as tpp,
        ):
            xt_sb = constp.tile([128, KCH, B], f32, tag="xt")
            nc.sync.dma_start(out=xt_sb[:], in_=xt_v[:])
            iv_sb = constp.tile([B, 1], f32, tag="iv")
            nc.sync.dma_start(out=iv_sb[:, 0], in_=iv_d[:])
            ident = constp.tile([128, 128], f32, tag="ident")
            nc.sync.dma_start(out=ident[:], in_=id_d[:])

            m_sb = stp.tile([B, 1], f32, tag="m")
            mnew = stp.tile([B, 1], f32, tag="mnew")
            cmax = stp.tile([B, 1], f32, tag="cmax")
            delta = stp.tile([B, 1], f32, tag="delta")
            resc = stp.tile([B, 1], f32, tag="resc")
            negb = stp.tile([B, 1], f32, tag="negb")
            s_sb = stp.tile([B, 1], f32, tag="s")
            ssum = stp.tile([B, 1], f32, tag="ssum")
            ms_sb = stp.tile([B, 2], f32, tag="ms")
            acc_sb = stp.tile([B, D], f32, tag="acc")

            for si, (c0, ncn) in enumerate(supers):
                ytS = ytp.tile([128, ncn, KCH, 128], f32, tag="yts")
                nc.sync.dma_start(out=ytS[:], in_=yt_v[:, ds(c0, ncn)])
                ybS = ybp.tile([128, ncn, D], bf16, tag="ybs")
                nc.sync.dma_start(out=ybS[:], in_=yb_v[:, ds(c0, ncn)])

                L_sb = lwp.tile([B, ncn, 128], f32, tag="L")
                # ---- phase 1: U chunks, transposed orientation ----
                for ci in range(ncn):
                    lt_ps = tpp.tile([128, B], f32, tag="tp")
                    for k in range(KCH):
                        nc.tensor.matmul(
                            lt_ps[:],
                            ytS[:, ci, k, :],
                            xt_sb[:, k, :],
                            start=(k == 0),
                            stop=(k == KCH - 1),
                        )
                    lt_sb = wtp.tile([128, B], f32, tag="lt")
                    nc.vector.tensor_copy(lt_sb[:], lt_ps[:])
                    lc_ps = tpp.tile([B, 128], f32, tag="tp")
                    nc.tensor.transpose(lc_ps[:], lt_sb[:], ident[:])
                    nc.vector.tensor_copy(L_sb[:, ci, :], lc_ps[:])

                # ---- phase 2: online softmax stats for this super ----
                nc.vector.tensor_reduce(
                    out=cmax[:], in_=L_sb[:], axis=AXY, op=mx
                )
                if si == 0:
                    nc.vector.tensor_copy(m_sb[:], cmax[:])
                else:
                    nc.vector.tensor_tensor(
                        out=mnew[:], in0=m_sb[:], in1=cmax[:], op=mx
                    )
                    nc.vector.tensor_tensor(
                        out=delta[:], in0=m_sb[:], in1=mnew[:], op=sub
                    )
                    nc.scalar.activation(
                        out=resc[:], in_=delta[:], func=Exp, scale=iv_sb[:]
                    )
                    nc.vector.tensor_copy(m_sb[:], mnew[:])
                nc.vector.tensor_scalar(
                    out=negb[:],
                    in0=m_sb[:],
                    scalar1=iv_sb[:],
                    scalar2=-1.0,
                    op0=mult,
                    op1=mult,
                )
                W_sb = lwp.tile([B, ncn, 128], f32, tag="W")
                nc.scalar.activation(
                    out=W_sb[:],
                    in_=L_sb[:],
                    func=Exp,
                    bias=negb[:],
                    scale=iv_sb[:],
                    accum_out=ssum[:],
                )
                if si == 0:
                    nc.vector.tensor_copy(s_sb[:], ssum[:])
                else:
                    nc.vector.tensor_scalar(
                        out=s_sb[:], in0=s_sb[:], scalar1=resc[:],
                        scalar2=None, op0=mult
                    )
                    nc.vector.tensor_tensor(
                        out=s_sb[:], in0=s_sb[:], in1=ssum[:], op=add
                    )

                # ---- phase 3: acc_ps += W^T @ Y (bf16), then drain ----
                acc_ps = accpsp.tile([B, D], f32, tag="accps")
                for ci in range(ncn):
                    wt_ps = tpp.tile([128, B], f32, tag="tp")
                    nc.tensor.transpose(
                        wt_ps[:], W_sb[:, ci, :], ident[0:B, 0:B]
                    )
                    wt_sb = wtp.tile([128, B], bf16, tag="wt")
                    nc.vector.tensor_copy(wt_sb[:], wt_ps[:])
                    for g in range(DG):
                        nc.tensor.matmul(
                            acc_ps[:, ts(g, 512)],
                            wt_sb[:],
                            ybS[:, ci, ts(g, 512)],
                            start=(ci == 0),
                            stop=(ci == ncn - 1),
                        )
                for g in range(DG):
                    if si == 0:
                        nc.vector.tensor_copy(
                            acc_sb[:, ts(g, 512)], acc_ps[:, ts(g, 512)]
                        )
                    else:
                        nc.vector.tensor_scalar(
                            out=acc_sb[:, ts(g, 512)],
                            in0=acc_sb[:, ts(g, 512)],
                            scalar1=resc[:],
                            scalar2=None,
                            op0=mult,
                        )
                        nc.vector.tensor_tensor(
                            out=acc_sb[:, ts(g, 512)],
                            in0=acc_sb[:, ts(g, 512)],
                            in1=acc_ps[:, ts(g, 512)],
                            op=add,
                        )

            # ---- outputs: acc, M = iv*m (logit-domain max), s ----
            nc.vector.tensor_scalar(
                out=ms_sb[:, 0:1], in0=negb[:], scalar1=-1.0,
                scalar2=None, op0=mult
            )
            nc.vector.tensor_copy(ms_sb[:, 1:2], s_sb[:])
            nc.sync.dma_start(out=out_d[:, 0:D], in_=acc_sb[:])
            nc.sync.dma_start(out=out_d[:, D : D + 2], in_=ms_sb[:])

    nc.compile()
    return nc


def _get_program():
    if "nc" not in _PROGRAM_CACHE:
        _PROGRAM_CACHE["nc"] = _build_program()
    return _PROGRAM_CACHE["nc"]


def _prep_inputs(x, sigma, Y):
    import ml_dtypes

    bf16 = ml_dtypes.bfloat16
    xf = np.ascontiguousarray(x.reshape(B, D)).astype(np.float32)
    Yf = np.ascontiguousarray(Y.reshape(N, D)).astype(np.float32)
    sigma = sigma.astype(np.float32)
    inv_var = (1.0 / (sigma * sigma)).astype(np.float32)

    xt = np.zeros((DP, B), dtype=np.float32)
    xt[:D, :] = xf.T
    xt[D, :] = -0.5
    xt_img = np.ascontiguousarray(
        xt.reshape(KCH, 128, B).transpose(1, 0, 2)
    ).reshape(128, KCH * B)

    ident = np.eye(128, dtype=np.float32)

    y2 = np.einsum("nd,nd->n", Yf, Yf).astype(np.float32)

    per_core = []
    for c in range(NCORES):
        sl = slice(c * NSH, (c + 1) * NSH)
        Yc = Yf[sl]
        # yt: [NP, DP] augmented, then tiled [128p][NK kk][KCH k][128 j]
        Yaug = np.zeros((NP, DP), dtype=np.float32)
        Yaug[:NSH, :D] = Yc
        Yaug[:NSH, D] = y2[sl]
        Yaug[NSH:, D] = Y2_PAD
        yt_img = np.ascontiguousarray(
            Yaug.reshape(NK, 128, KCH, 128).transpose(3, 0, 2, 1)
        ).reshape(128, NK * KCH * 128)
        # yb: [NP, D] bf16, row-permuted [128 j][NK kk][D]
        Ypad = np.zeros((NP, D), dtype=bf16)
        Ypad[:NSH] = Yc.astype(bf16)
        yb_img = np.ascontiguousarray(
            Ypad.reshape(NK, 128, D).transpose(1, 0, 2)
        ).reshape(128, NK * D)
        per_core.append(
            {
                "xt": xt_img,
                "iv": inv_var,
                "ident": ident,
                "yt": yt_img,
                "yb": yb_img,
            }
        )
    return per_core


def _merge(results):
    # per-core outputs: out[:, :D]=acc, out[:, D]=M (iv*max), out[:, D+1]=s
    ms = np.stack([r["out"][:, D] for r in results])       # [NCORES, B]
    ss = np.stack([r["out"][:, D + 1] for r in results])   # [NCORES, B]
    accs = np.stack([r["out"][:, :D] for r in results])    # [NCORES, B, D]
    m_glob = ms.max(axis=0)                                # [B]
    corr = np.exp(ms - m_glob[None, :])                    # [NCORES, B]
    s_tot = (ss * corr).sum(axis=0)                        # [B]
    acc_tot = np.einsum("cb,cbd->bd", corr, accs)          # [B, D]
    return acc_tot / s_tot[:, None]


def kernel(x, sigma, Y):
    from concourse.bass_utils import run_bass_kernel_spmd

    nc = _get_program()
    in_maps = _prep_inputs(np.asarray(x), np.asarray(sigma), np.asarray(Y))
    res = run_bass_kernel_spmd(nc, in_maps, list(range(NCORES)))
    out = _merge(res.results)
    return out.reshape(B, CC, HH, WW).astype(np.float32)


if __name__ == "__main__":
    rng = np.random.default_rng(0)
    x = rng.standard_normal((B, CC, HH, WW), dtype=np.float32)
    sigma = (rng.random(B, dtype=np.float32) * 1.9 + 0.1).astype(np.float32)
    Y = rng.standard_normal((N, CC, HH, WW), dtype=np.float32)
    out = kernel(x=x, sigma=sigma, Y=Y)
    print("out", out.shape, out.dtype, float(np.abs(out).mean()))
